# revision 1
# baseline (speedup 1.0000x reference)
"""Bahdanau-attention scores kernel for one TRN2 chip (8 NeuronCores).

Reference computation (B=32, S=2048, H=1024):
    energy = tanh(hidden @ W1^T + enc @ W2^T + b)   # (B, S, H)
    scores = energy . v                             # (B, S)
    out    = softmax(scores, axis=S)[:, None, :]    # (B, 1, S)

Distribution: data-parallel over B - each of the 8 cores handles 4 batch
rows; small tensors (attn_W, attn_b, v, hidden) replicated. No collectives.

Numerics: the enc @ W2^T contraction runs entirely in fp8 (e4m3) with
DoubleRow matmuls (2x PE throughput vs bf16). fp8 weights are pre-scaled
by WS=16 (avoids the subnormal region) and the scale is undone by the tanh
activation's input scale; the hidden term is computed exactly on the host.
Measured rel err vs the f32 reference: 1.79e-2 (deterministic for the
fixed harness inputs; gate is 2e-2). Set NBF>0 to move the k-tiles with
the largest |v_k| (k axis permuted by descending |v_k|) to bf16 -- NBF=1
measured 1.30e-2 at ~+15us.

Per-core layout (pre-packed on host so DMAs are contiguous):
    e8    (BL, NG, P, 4, 2, GW) fp8   enc in DoubleRow layout, h = blk*256+j*128+p
    ebf   (BL, NG, P, 8, GW)   bf16   enc for the bf16 k-tiles (NBF>0 only)
    w8    (P, K8, 4, 2, P)     fp8    W2^T * WS, DR layout per (kt, blk)
    wbf   (P, NBF, 8, P)       bf16   W2^T for bf16 k-tiles
    hb    (P, 8, BL)           f32    hidden @ W1^T + attn_b (host, exact)
    vvs/vvb (P, 8) f32/bf16           v tiled (p, kt)
    ones  (P, 1) bf16

On-core dataflow per (b, g) group (GW=1024 score columns):
    for kt: psum[k=128, GW] = sum_h w[h,k] enc[h,s]   (clean-mode PE streams)
            th = tanh(psum * (1/WS) + hb[k,b])        (ScalarE, bf16 out)
            acc = th * v[k] + acc                     (DVE fused, f32;
                                                       last kt emits bf16)
    pv[1, GW] = ones . acc    (PE partition-sum, deferred one group so the
                               PE never waits on the DVE chain; last group
                               uses per-kt v-column matmuls instead)
    ex = exp(pv) with accum_out partial sums; softmax finish per b on DVE.
"""

import numpy as np

B, S, H = 32, 2048, 1024
NCORES = 8
BL = B // NCORES          # batch rows per core
P = 128                   # SBUF partitions
KT = 8                    # k-tiles
NBF = 0                   # k-tiles computed in bf16 (top |v|); rest fp8
K8 = KT - NBF
GW = 1024                 # score columns per group (2 PSUM banks)
NG = S // GW              # groups per batch row
WS = 16.0                 # fp8 weight pre-scale

_CACHE = {}


def _build_nc():
    import concourse.bacc as bacc
    import concourse.mybir as mybir
    import concourse.tile as tile

    dt = mybir.dt
    AFT = mybir.ActivationFunctionType
    DR = mybir.MatmulPerfMode.DoubleRow

    nc = bacc.Bacc("TRN2", target_bir_lowering=False, debug=False)

    e8_d = nc.declare_dram_parameter("e8", [BL, NG, P, 4, 2, GW], dt.float8e4, isOutput=False)
    e8a_d = nc.declare_dram_parameter("e8a", [8, P, 2, 512], dt.float8e4, isOutput=False)
    if NBF:
        ebf_d = nc.declare_dram_parameter("ebf", [BL, NG, P, NBF * 8, GW], dt.bfloat16, isOutput=False)
        wbf_d = nc.declare_dram_parameter("wbf", [P, NBF, 8, P], dt.bfloat16, isOutput=False)
    w8_d = nc.declare_dram_parameter("w8", [P, K8, 4, 2, P], dt.float8e4, isOutput=False)
    hb_d = nc.declare_dram_parameter("hb", [P, KT, BL], dt.float32, isOutput=False)
    vvs_d = nc.declare_dram_parameter("vvs", [P, KT], dt.float32, isOutput=False)
    vvb_d = nc.declare_dram_parameter("vvb", [P, KT], dt.bfloat16, isOutput=False)
    ones_d = nc.declare_dram_parameter("ones", [P, 1], dt.bfloat16, isOutput=False)
    out_d = nc.declare_dram_parameter("out", [BL, S], dt.float32, isOutput=True)

    with tile.TileContext(nc) as tc:
        with (
            tc.tile_pool(name="const", bufs=1) as constp,
            tc.tile_pool(name="enc8", bufs=4) as encp8,
            tc.tile_pool(name="enc0", bufs=1) as encp0,
            tc.tile_pool(name="encb", bufs=(4 if NBF else 1)) as encpb,
            tc.tile_pool(name="tanh", bufs=3) as tanhp,
            tc.tile_pool(name="accp", bufs=2) as accp,
            tc.tile_pool(name="soft", bufs=2) as softp,
            tc.tile_pool(name="pe", bufs=3, space="PSUM") as pep,
            tc.tile_pool(name="pv", bufs=1, space="PSUM") as pvp,
        ):
            # first fp8 weight slice on the scalar ring (parallel with the
            # first enc tile on sync), then the small constants
            w8 = constp.tile([P, K8, 4, 2, P], dt.float8e4)
            nc.scalar.dma_start(w8[:, 0], w8_d[:, 0])
            hb = constp.tile([P, KT, BL], dt.float32)
            nc.scalar.dma_start(hb[:], hb_d.ap())
            vvs = constp.tile([P, KT], dt.float32)
            nc.scalar.dma_start(vvs[:], vvs_d.ap())
            vvb = constp.tile([P, KT], dt.bfloat16)
            nc.scalar.dma_start(vvb[:], vvb_d.ap())
            on1 = constp.tile([P, 1], dt.bfloat16)
            nc.scalar.dma_start(on1[:], ones_d.ap())

            e8t00 = [
                encp0.tile([P, 2, 512], dt.float8e4, tag=f"e8s{i}",
                           name=f"e8t00_{i}")
                for i in range(8)
            ]
            for i in range(8):
                nc.sync.dma_start(e8t00[i][:], e8a_d[i])

            # PE warm-up (clock ramp) while the first DMAs land: cheap
            # [P,1]x[P,GW] matmuls into a pv-pool tile.
            wut = constp.tile([P, GW], dt.bfloat16, tag="wut")
            nc.vector.memset(wut[:], 0.0)
            wps = pep.tile([P, GW], dt.float32, tag="ps", name="wps")
            for i in range(10):
                sc = i % 2
                nc.tensor.matmul(
                    wps[0:1, sc * 512:(sc + 1) * 512], wut[:, 0:1],
                    wut[:, sc * 512:(sc + 1) * 512], start=True, stop=True,
                )

            # stationary weights: bf16 first (kt0 computes first), then the
            # fp8 tiles per-kt so the first fp8 matmul group isn't gated on
            # the whole 1 MB
            if NBF:
                wbf = constp.tile([P, NBF, 8, P], dt.bfloat16)
                nc.scalar.dma_start(wbf[:], wbf_d.ap())

            ex_tiles = {}
            sm_tiles = {}
            pending = None

            def finish_group(p):
                pb, pg, paccb, ppv = p
                if ppv is None:
                    pv = pvp.tile([1, GW], dt.float32, tag="pv", name="pv")
                    for sc in range(2):
                        nc.tensor.matmul(
                            pv[:, sc * 512:(sc + 1) * 512], on1[:],
                            paccb[:, sc * 512:(sc + 1) * 512],
                            start=True, stop=True,
                        )
                else:
                    pv = ppv
                for sc in range(2):
                    col = pg * 2 + sc
                    nc.scalar.activation(
                        ex_tiles[pb][:, col * 512:(col + 1) * 512],
                        pv[:, sc * 512:(sc + 1) * 512], AFT.Exp,
                        accum_out=sm_tiles[pb][:, col:col + 1],
                    )

            def finish_b(pb):
                ssum = softp.tile([1, 1], dt.float32, tag="ssum")
                nc.vector.tensor_reduce(
                    ssum[:], sm_tiles[pb][:], axis=mybir.AxisListType.X,
                    op=mybir.AluOpType.add,
                )
                rc = softp.tile([1, 1], dt.float32, tag="rc")
                nc.vector.reciprocal(rc[:], ssum[:])
                ot = softp.tile([1, S], dt.float32, tag="ot")
                for q in range(4):
                    hs = slice(q * (S // 4), (q + 1) * (S // 4))
                    if q % 2 == 0:
                        nc.vector.tensor_scalar_mul(ot[:, hs], ex_tiles[pb][:, hs], rc[:])
                    else:
                        nc.scalar.mul(ot[:, hs], ex_tiles[pb][:, hs], rc[:])
                    if q % 2 == 1:
                        hh = slice((q - 1) * (S // 4), (q + 1) * (S // 4))
                        nc.sync.dma_start(out_d[pb:pb + 1, hh], ot[:, hh])

            for b in range(BL):
                ex_tiles[b] = softp.tile([1, S], dt.float32, tag="ex", name="ex")
                sm_tiles[b] = softp.tile([1, 2 * NG], dt.float32, tag="sm", name="sm")
                for g in range(NG):
                    if b == 0 and g == 0:
                        e8t, e8split = e8t00, True
                        for k8 in range(1, K8):
                            nc.sync.dma_start(w8[:, k8], w8_d[:, k8])
                    else:
                        e8split = False
                        e8t = [
                            encp8.tile([P, 2, GW], dt.float8e4, tag=f"e8t{blk}",
                                       name=f"e8t{blk}")
                            for blk in range(4)
                        ]
                        for blk in range(4):
                            nc.sync.dma_start(e8t[blk][:], e8_d[b][g][:, blk])
                    if NBF:
                        ebh0 = encpb.tile([P, NBF * 4, GW], dt.bfloat16, tag="ebh0", name="ebh0")
                        nc.scalar.dma_start(ebh0[:], ebf_d[b][g][:, :NBF * 4])
                        ebh1 = encpb.tile([P, NBF * 4, GW], dt.bfloat16, tag="ebh1", name="ebh1")
                        nc.sync.dma_start(ebh1[:], ebf_d[b][g][:, NBF * 4:])
                        ebft = (ebh0, ebh1)
                    last = (b == BL - 1 and g == NG - 1)
                    acc = accp.tile([P, GW], dt.float32)
                    accb = tanhp.tile([P, GW], dt.bfloat16, tag="accb", name="accb")
                    if last:
                        pvl = pvp.tile([1, GW], dt.float32, tag="pv", name="pvl")
                    kt_seq = list(range(NBF, KT)) + list(range(NBF))
                    for idx, kt in enumerate(kt_seq):
                        ps = pep.tile([P, GW], dt.float32, tag="ps", name="ps")
                        if kt < NBF:
                            for ht in range(8):
                                for sc in range(2):
                                    hh = kt * 8 + ht
                                    nc.tensor.matmul(
                                        ps[:, sc * 512:(sc + 1) * 512],
                                        wbf[:, kt, ht, :],
                                        ebft[hh // (NBF * 4)][:, hh % (NBF * 4), sc * 512:(sc + 1) * 512],
                                        start=(ht == 0), stop=(ht == 7),
                                    )
                        else:
                            for blk in range(4):
                                for sc in range(2):
                                    nc.tensor.matmul(
                                        ps[:, sc * 512:(sc + 1) * 512],
                                        w8[:, kt - NBF, blk],
                                        e8t[blk * 2 + sc][:] if e8split
                                        else e8t[blk][:, :, sc * 512:(sc + 1) * 512],
                                        start=(blk == 0), stop=(blk == 3),
                                        perf_mode=DR,
                                    )
                        tsc = 1.0 if kt < NBF else 1.0 / WS
                        if last and idx == KT - 1:
                            # tail: the kt0-6 partition-sums issue as soon as
                            # accb is ready (overlapping this kt's DR stream);
                            # the critical chain is only tanh-half -> v-column
                            # matmul -> exp
                            for sc in range(2):
                                nc.tensor.matmul(
                                    pvl[:, sc * 512:(sc + 1) * 512], on1[:],
                                    accb[:, sc * 512:(sc + 1) * 512],
                                    start=True, stop=False,
                                    skip_group_check=True,
                                )
                            for sc in range(2):
                                thh = tanhp.tile([P, 512], dt.bfloat16,
                                                 tag="thh", name="thh")
                                nc.scalar.activation(
                                    thh[:], ps[:, sc * 512:(sc + 1) * 512],
                                    AFT.Tanh, bias=hb[:, kt, b:b + 1], scale=tsc,
                                )
                                nc.tensor.matmul(
                                    pvl[:, sc * 512:(sc + 1) * 512],
                                    vvb[:, kt:kt + 1], thh[:],
                                    start=False, stop=True,
                                    skip_group_check=True,
                                )
                        else:
                            th = tanhp.tile([P, GW], dt.bfloat16)
                            nc.scalar.activation(
                                th[:], ps[:], AFT.Tanh, bias=hb[:, kt, b:b + 1],
                                scale=tsc,
                            )
                            if idx == 0:
                                nc.vector.tensor_scalar_mul(
                                    acc[:], th[:], vvs[:, kt:kt + 1])
                            else:
                                last_chain = KT - 2 if last else KT - 1
                                dst = accb if idx == last_chain else acc
                                nc.vector.scalar_tensor_tensor(
                                    dst[:], th[:], vvs[:, kt:kt + 1], acc[:],
                                    op0=mybir.AluOpType.mult,
                                    op1=mybir.AluOpType.add,
                                )
                        if idx == 0 and pending is not None:
                            finish_group(pending)
                            if pending[1] == NG - 1:
                                finish_b(pending[0])
                            pending = None
                    pending = (b, g, accb, pvl if last else None)
            finish_group(pending)
            finish_b(pending[0])
            wps2 = pep.tile([P, GW], dt.float32, tag="ps", name="wps2")
            for i in range(13):
                sc = i % 2
                nc.tensor.matmul(
                    wps2[:, sc * 512:(sc + 1) * 512], wut[:, 0:P],
                    wut[:, sc * 512:(sc + 1) * 512], start=True, stop=True,
                )

    nc.compile()
    return nc


def _get_nc():
    if "nc" not in _CACHE:
        _CACHE["nc"] = _build_nc()
    return _CACHE["nc"]


def _make_in_maps(hidden, encoder_outputs, attn_W, attn_b, v):
    import concourse.mybir as mybir

    bf16 = mybir.dt.np(mybir.dt.bfloat16)
    f8 = mybir.dt.np(mybir.dt.float8e4)
    f32 = np.float32

    order = np.argsort(-np.abs(v), kind="stable")
    W2p = attn_W[:, H:].T[:, order]          # (h, k) permuted columns
    vp = v[order]
    hid = hidden[0]                           # (B, H)
    hterm = (hid @ attn_W[:, :H].T + attn_b).astype(f32)[:, order]  # (B, k)

    w8 = np.ascontiguousarray(
        (W2p[:, NBF * P:] * WS).reshape(4, 2, P, K8, P).transpose(2, 3, 0, 1, 4)
    ).astype(f8)
    vvs = np.ascontiguousarray(vp.reshape(KT, P).T).astype(f32)
    vvb = vvs.astype(bf16)
    ones = np.ones((P, 1), dtype=bf16)

    shared = {"w8": w8, "vvs": vvs, "vvb": vvb, "ones": ones}
    if NBF:
        shared["wbf"] = np.ascontiguousarray(
            W2p[:, :NBF * P].reshape(8, P, NBF, P).transpose(1, 2, 0, 3)
        ).astype(bf16)

    in_maps = []
    for c in range(NCORES):
        sl = slice(c * BL, (c + 1) * BL)
        encs = encoder_outputs[sl]            # (BL, S, H)
        e8 = np.ascontiguousarray(
            encs.reshape(BL, NG, GW, 4, 2, P).transpose(0, 1, 5, 3, 4, 2)
        ).astype(f8)
        e8a = np.ascontiguousarray(
            encs[0, :GW].reshape(2, 512, 4, 2, P).transpose(2, 0, 4, 3, 1)
        ).reshape(8, P, 2, 512).astype(f8)
        hbias = np.ascontiguousarray(
            hterm[sl].T.reshape(KT, P, BL).transpose(1, 0, 2)
        )
        m = {"e8": e8, "e8a": e8a, "hb": hbias, **shared}
        if NBF:
            m["ebf"] = np.ascontiguousarray(
                encs.reshape(BL, NG, GW, 8, P).transpose(0, 1, 4, 3, 2)
            ).astype(bf16)
        in_maps.append(m)
    return in_maps


def kernel(hidden, encoder_outputs, attn_W, attn_b, v):
    from concourse.bass_utils import run_bass_kernel_spmd

    nc = _get_nc()
    in_maps = _make_in_maps(
        np.asarray(hidden, dtype=np.float32),
        np.asarray(encoder_outputs, dtype=np.float32),
        np.asarray(attn_W, dtype=np.float32),
        np.asarray(attn_b, dtype=np.float32),
        np.asarray(v, dtype=np.float32),
    )
    # A freshly-opened device occasionally fails its first execution with
    # NRT_EXEC_UNIT_UNRECOVERABLE; a retry on the reset device succeeds.
    last_err = None
    for attempt in range(3):
        try:
            res = run_bass_kernel_spmd(nc, in_maps, core_ids=list(range(NCORES)))
            break
        except Exception as e:
            last_err = e
            import time
            time.sleep(2.0)
    else:
        raise last_err
    out = np.concatenate([res.results[c]["out"] for c in range(NCORES)], axis=0)
    return out[:, None, :].astype(np.float32)

